# revision 1
# baseline (speedup 1.0000x reference)
"""Self-contained Trainium2 kernel for nn_BanzhafModule (conv1 -> self-attention -> conv2).

Data-parallel over 8 NeuronCores: each core processes 4 of the 32 (b*a) batch
elements end-to-end; no collectives. Heavy matmuls run on TensorE in fp32r
(conv1/QKV/scores/conv2-H) and bf16 (attn*V / conv2-O); softmax uses an exact
per-row max computed from a second scores pass, injected as a K=1 matmul.
All spatial tensors stay in compact [channel, 1024] layout; conv zero-padding
is realized by host-side im2col (conv1) and clipped-window adds (conv2).
"""

import numpy as np

E = 4          # batch elements per core
NCORES = 8
IMG = 32       # t = v = 32
L = IMG * IMG  # 1024 tokens
P = 512        # planes

_TAPS = [(dy, dx) for dy in range(3) for dx in range(3)]

_built = {}


def _build_nc():
    import os
    STAGE = int(os.environ.get("KSTAGE", "99"))
    import concourse.mybir as mybir
    from concourse import bacc
    from concourse.tile import TileContext
    from concourse.masks import make_identity

    f32, f32r, bf16 = mybir.dt.float32, mybir.dt.float32r, mybir.dt.bfloat16
    AF = mybir.ActivationFunctionType
    ALU = mybir.AluOpType
    AX = mybir.AxisListType

    nc = bacc.Bacc("TRN2", target_bir_lowering=False, debug=False, num_devices=NCORES)

    i_xcol = nc.dram_tensor("xcol", [E, 9, L], f32, kind="ExternalInput")
    i_w1 = nc.dram_tensor("W1c", [9, P], f32, kind="ExternalInput")
    i_q = nc.dram_tensor("Qm", [128, 4, P], f32, kind="ExternalInput")
    i_k = nc.dram_tensor("Km", [128, 4, P], f32, kind="ExternalInput")
    i_v = nc.dram_tensor("Vm", [128, 4, P], f32, kind="ExternalInput")
    i_w2 = nc.dram_tensor("W2m", [128, 4, 9], f32, kind="ExternalInput")
    i_b1 = nc.dram_tensor("b1v", [128, 4], f32, kind="ExternalInput")
    i_b2 = nc.dram_tensor("b2v", [1, 1], f32, kind="ExternalInput")
    o_out = nc.dram_tensor("out", [E, L], f32, kind="ExternalOutput")

    ones_col_d = nc.inline_tensor(np.ones((128, 1), np.float32), name="ones_col")
    ones_row_d = nc.inline_tensor(np.ones((1, 128), np.float32), name="ones_row")

    with TileContext(nc) as tc:
        with (
            tc.tile_pool(name="wts", bufs=1) as wts,
            tc.tile_pool(name="hp", bufs=2) as hp,
            tc.tile_pool(name="qp", bufs=2) as qp,
            tc.tile_pool(name="kp", bufs=2) as kp,
            tc.tile_pool(name="vp", bufs=2) as vp,
            tc.tile_pool(name="ep", bufs=1) as ep,
            tc.tile_pool(name="op", bufs=1) as op_,
            tc.tile_pool(name="xp", bufs=1) as xp,
            tc.tile_pool(name="stg", bufs=1) as stg,
            tc.tile_pool(name="msc", bufs=1) as msc,
            tc.tile_pool(name="fin", bufs=1) as fin,
            tc.tile_pool(name="pmm", bufs=3, space="PSUM") as pmm,
            tc.tile_pool(name="ptp", bufs=2, space="PSUM") as ptp,
            tc.tile_pool(name="xm", bufs=2) as xm,
        ):
            # ---- weights / constants (persistent) ----
            def load_r(name, src_ap, shape):
                stage = stg.tile(shape, f32, tag="wstage")
                nc.sync.dma_start(stage[:], src_ap)
                dst = wts.tile(shape, f32r, tag=name)
                nc.vector.tensor_copy(dst[:], stage[:])
                return dst

            w1c = load_r("w1c", i_w1.ap(), [9, P])
            b1t = wts.tile([128, 4], f32)
            nc.sync.dma_start(b1t[:], i_b1.ap())
            prefetch = {}
            xcf0 = xp.tile([9, L], f32, tag="xcolf", name="xcf0")
            nc.sync.dma_start(xcf0[:], i_xcol.ap()[0])
            prefetch[0] = xcf0
            def load_r4(name, src_ap):
                dst = wts.tile([128, 4, P], f32r, tag=name, name=name)
                for dk in range(4):
                    stage = stg.tile([128, 1, P], f32, tag="wstage4", name=f"{name}s{dk}")
                    nc.sync.dma_start(stage[:], src_ap[:, dk:dk + 1, :])
                    nc.vector.tensor_copy(dst[:, dk:dk + 1, :], stage[:])
                return dst

            qm = load_r4("qm", i_q.ap())
            km = load_r4("km", i_k.ap())
            vm = load_r4("vm", i_v.ap())
            w2f = load_r("w2f", i_w2.ap(), [128, 4, 9])
            onc = wts.tile([128, 1], f32)
            nc.sync.dma_start(onc[:], ones_col_d.ap())
            oncb = wts.tile([128, 1], bf16)
            nc.vector.tensor_copy(oncb[:], onc[:])
            w2b = wts.tile([128, 4, 9], bf16)
            nc.scalar.copy(w2b[:], w2f[:])
            ident = wts.tile([128, 128], f32)
            make_identity(nc, ident[:])
            identb = wts.tile([128, 128], bf16)
            make_identity(nc, identb[:])

            b2t = wts.tile([1, 1], f32)
            nc.sync.dma_start(b2t[:], i_b2.ap())
            p9sh = fin.tile([9, E, L], bf16)
            nc.gpsimd.memset(p9sh[:], 0.0)

            state = {}

            def conv1_qkv(e):
                xcf = prefetch.pop(e, None)
                if xcf is None:
                    xcf = xp.tile([9, L], f32, tag="xcolf")
                    nc.sync.dma_start(xcf[:], i_xcol.ap()[e])
                xc = xp.tile([9, L], f32r, tag="xcol")
                nc.scalar.copy(xc[:], xcf[:])
                # conv1: h[p, l] = relu(sum_j W1c[j, p] * xcol[j, l] + b1[p])
                ht = hp.tile([128, 4, L], f32r, tag="H")
                for ck in range(4):
                    ps = pmm.tile([128, 1024], f32, tag="pmm")
                    for lg in range(2):
                        nc.tensor.matmul(
                            ps[:, lg * 512:(lg + 1) * 512],
                            w1c[:, ck * 128:(ck + 1) * 128],
                            xc[:, lg * 512:(lg + 1) * 512],
                            start=True, stop=True,
                        )
                    nc.scalar.activation(
                        ht[:, ck, :], ps[:], AF.Relu, bias=b1t[:, ck:ck + 1]
                    )
                # q/k projections (fp32r), vv projection (to bf16)
                qt = qp.tile([128, 4, L], f32r, tag="qT")
                kt = kp.tile([128, 4, L], f32r, tag="kT")
                for dst, wm in ((qt, qm), (kt, km)):
                    for nck in range(4):
                        ps = pmm.tile([128, 1024], f32, tag="pmm")
                        for lg in range(2):
                            for dk in range(4):
                                nc.tensor.matmul(
                                    ps[:, lg * 512:(lg + 1) * 512],
                                    wm[:, dk, nck * 128:(nck + 1) * 128],
                                    ht[:, dk, lg * 512:(lg + 1) * 512],
                                    start=(dk == 0), stop=(dk == 3),
                                )
                        if nck % 2 == 0:
                            nc.scalar.copy(dst[:, nck, :], ps[:])
                        else:
                            nc.vector.tensor_copy(dst[:, nck, :], ps[:])
                vv = vp.tile([128, 8, 512], bf16, tag="vv")
                for lc in range(8):
                    ps = pmm.tile([128, 1024], f32, tag="pmm")
                    for dk in range(4):
                        nc.tensor.matmul(
                            ps[:, 0:512],
                            ht[:, dk, lc * 128:(lc + 1) * 128],
                            vm[:, dk, :],
                            start=(dk == 0), stop=(dk == 3),
                        )
                    nc.vector.tensor_copy(vv[:, lc, :], ps[:, 0:512])
                state[e] = (ht, qt, kt, vv)

            def attention(e):
                ht, qt, kt, vv = state[e]
                if STAGE < 2:
                    if e + 1 < E:
                        conv1_qkv(e + 1)
                    return
                # ---- scores in M-layout; exp with fused -max bias and rowsum;
                #      PE-transpose each 128x128 attn tile into T-layout ----
                nmcol = msc.tile([128, 8], f32, tag="nmcol")
                rscol = msc.tile([128, 8], f32, tag="rscol")
                et = ep.tile([128, 8, L], bf16, tag="eT")
                for lc in range(8):
                    ps = pmm.tile([128, 1024], f32, tag="pmm")
                    for mg in range(2):
                        for ncx in range(4):
                            nc.tensor.matmul(
                                ps[:, mg * 512:(mg + 1) * 512],
                                qt[:, ncx, lc * 128:(lc + 1) * 128],
                                kt[:, ncx, mg * 512:(mg + 1) * 512],
                                start=(ncx == 0), stop=(ncx == 3),
                            )
                    nc.vector.tensor_reduce(
                        nmcol[:, lc:lc + 1], ps[:], axis=AX.X, op=ALU.max, negate=True
                    )
                    expm = xm.tile([128, 1024], bf16, tag="expM")
                    nc.scalar.activation(
                        expm[:], ps[:], AF.Exp,
                        bias=nmcol[:, lc:lc + 1],
                        accum_out=rscol[:, lc:lc + 1],
                    )
                    ptr = ptp.tile([128, 1024], bf16, tag="ptr")
                    for mc in range(8):
                        nc.tensor.transpose(
                            ptr[:, mc * 128:(mc + 1) * 128],
                            expm[:, mc * 128:(mc + 1) * 128],
                            identb[:],
                        )
                    for mc in range(0, 8, 2):
                        dst = et[:, mc:mc + 2, lc * 128:(lc + 1) * 128]
                        srcp = ptr[:, mc * 128:(mc + 2) * 128].rearrange(
                            "p (c w) -> p c w", c=2
                        )
                        if mc % 4 == 0:
                            nc.scalar.copy(dst, srcp)
                        else:
                            nc.vector.tensor_copy(dst, srcp)

                if STAGE < 3:
                    if e + 1 < E:
                        conv1_qkv(e + 1)
                    return
                if STAGE < 4:
                    if e + 1 < E:
                        conv1_qkv(e + 1)
                    return
                # ---- reciprocal of rowsums, then fan out as a [9, L] row set ----
                rcol = msc.tile([128, 8], f32, tag="rcol")
                nc.vector.reciprocal(rcol[:], rscol[:])
                pt = ptp.tile([8, 128], f32, tag="ptr", name="pt")
                nc.tensor.transpose(pt[:], rcol[:], ident[:])
                rc8 = msc.tile([8, 128], f32, tag="rc8")
                nc.vector.tensor_copy(rc8[:], pt[:])
                rcc = msc.tile([1, L], f32, tag="rcc")
                for c in range(8):
                    nc.sync.dma_start(rcc[0:1, 128 * c:128 * (c + 1)], rc8[c:c + 1, :])
                rbc9 = msc.tile([9, L], f32, tag="rbc9")
                for c in range(9):
                    nc.sync.dma_start(rbc9[c:c + 1, :], rcc[0:1, :])

                if STAGE < 5:
                    if e + 1 < E:
                        conv1_qkv(e + 1)
                    return
                # ---- O^T = vv^T @ expS^T (unnormalized), compact layout ----
                osc = op_.tile([128, 4, L], bf16, tag="Osc")
                for dc in range(4):
                    ps = pmm.tile([128, 1024], f32, tag="pmm")
                    for lg in range(2):
                        sl = slice(lg * 512, (lg + 1) * 512)
                        for mc in range(8):
                            nc.tensor.matmul(
                                ps[:, sl],
                                vv[:, mc, dc * 128:(dc + 1) * 128],
                                et[:, mc, sl],
                                start=(mc == 0), stop=(mc == 7),
                            )
                    nc.scalar.copy(osc[:, dc, :], ps[:])

                # next elem's prologue fills PE while conv2's DVE/DMA tail runs
                if e + 1 < E:
                    conv1_qkv(e + 1)
                if STAGE < 6:
                    return
                # ---- conv2 taps on compact layout: P9H (fp32r) + P9O (bf16) ----
                p9e = msc.tile([9, L], bf16, tag="p9e")
                for lg in range(2):
                    sl = slice(lg * 512, (lg + 1) * 512)
                    p9h = ptp.tile([9, 512], f32, tag="ptr", name="p9h")
                    p9o = ptp.tile([9, 512], f32, tag="ptr", name="p9o")
                    for ck in range(4):
                        nc.tensor.matmul(
                            p9h[:], w2f[:, ck, :], ht[:, ck, sl],
                            start=(ck == 0), stop=(ck == 3),
                        )
                    for ck in range(4):
                        nc.tensor.matmul(
                            p9o[:], w2b[:, ck, :], osc[:, ck, sl],
                            start=(ck == 0), stop=(ck == 3),
                        )
                    nc.vector.tensor_tensor(p9e[:, sl], p9o[:], rbc9[:, sl], ALU.mult)
                    nc.vector.tensor_tensor(p9e[:, sl], p9e[:, sl], p9h[:], ALU.add)
                if STAGE < 7:
                    return
                # scatter each tap row into its shifted, clipped window (DMA:
                # byte-addressed, so the unaligned partition bases are fine)
                for j, (dy, dx) in enumerate(_TAPS):
                    r0, r1 = max(0, 1 - dy), min(IMG, IMG + 1 - dy)
                    c0, c1 = max(0, 1 - dx), min(IMG, IMG + 1 - dx)
                    srcw = p9e[j:j + 1, :].rearrange("o (r w) -> o r w", w=IMG)[
                        :, r0 + dy - 1:r1 + dy - 1, c0 + dx - 1:c1 + dx - 1
                    ]
                    dstw = p9sh[j:j + 1, e, :].rearrange("o (r w) -> o r w", w=IMG)[
                        :, r0:r1, c0:c1
                    ]
                    nc.gpsimd.dma_start(dstw, srcw)
                if STAGE < 8:
                    return
                # sum the 9 tap rows on TensorE and add b2 on the way out
                acc1 = msc.tile([1, L], f32, tag="acc1")
                for lg in range(2):
                    sl = slice(lg * 512, (lg + 1) * 512)
                    psf = ptp.tile([1, 512], f32, tag="ptr", name="psf")
                    nc.tensor.matmul(
                        psf[:], oncb[0:9, 0:1], p9sh[0:9, e, sl],
                        start=True, stop=True,
                    )
                    nc.scalar.activation(
                        acc1[0:1, sl], psf[:], AF.Identity, bias=b2t[0:1, 0:1]
                    )
                if STAGE >= 9:
                    nc.sync.dma_start(o_out.ap()[e:e + 1, :], acc1[0:1, :])

            conv1_qkv(0)
            for e in range(E):
                attention(e)

    nc.compile()
    return nc


def _host_prep(x, W1, b1, Q, K, V, W2, b2):
    B = x.shape[0] * x.shape[1]
    xf = np.ascontiguousarray(x, np.float32).reshape(B, IMG, IMG)
    xpad = np.zeros((B, IMG + 2, IMG + 2), np.float32)
    xpad[:, 1:-1, 1:-1] = xf
    xcol = np.empty((B, 9, L), np.float32)
    for j, (dy, dx) in enumerate(_TAPS):
        xcol[:, j] = xpad[:, dy:dy + IMG, dx:dx + IMG].reshape(B, L)
    w1c = np.ascontiguousarray(np.asarray(W1, np.float32).reshape(P, 9).T)
    qm = np.ascontiguousarray(np.asarray(Q, np.float32).reshape(4, 128, P).transpose(1, 0, 2))
    km = np.ascontiguousarray(np.asarray(K, np.float32).reshape(4, 128, P).transpose(1, 0, 2))
    vm = np.ascontiguousarray(np.asarray(V, np.float32).reshape(4, 128, P).transpose(1, 0, 2))
    w2m = np.ascontiguousarray(np.asarray(W2, np.float32).reshape(P, 9).reshape(4, 128, 9).transpose(1, 0, 2))
    b1v = np.ascontiguousarray(np.asarray(b1, np.float32).reshape(4, 128).T)
    b2v = np.asarray(b2, np.float32).reshape(1, 1)
    return xcol, w1c, qm, km, vm, w2m, b1v, b2v


def kernel(x, W1, b1, Q, K, V, W2, b2):
    from concourse.bass_utils import run_bass_kernel_spmd

    xcol, w1c, qm, km, vm, w2m, b1v, b2v = _host_prep(x, W1, b1, Q, K, V, W2, b2)
    if "nc" not in _built:
        _built["nc"] = _build_nc()
    nc = _built["nc"]
    in_maps = []
    for c in range(NCORES):
        in_maps.append({
            "xcol": np.ascontiguousarray(xcol[E * c:E * (c + 1)]),
            "W1c": w1c, "Qm": qm, "Km": km, "Vm": vm,
            "W2m": w2m, "b1v": b1v, "b2v": b2v,
        })
    res = run_bass_kernel_spmd(nc, in_maps, core_ids=list(range(NCORES)))
    full = np.concatenate([res.results[c]["out"] for c in range(NCORES)], axis=0)
    return np.ascontiguousarray(
        full.reshape(x.shape[0], x.shape[1], IMG, IMG).astype(np.float32)
    )



# revision 10
# speedup vs baseline: 1.5592x; 1.5592x over previous
"""Self-contained Trainium2 kernel for nn_BanzhafModule (conv1 -> self-attention -> conv2).

Data-parallel over 8 NeuronCores: each core processes 4 of the 32 (b*a) batch
elements end-to-end; no collectives.

Algebraic restructure vs the straightforward formulation (all host-precomputed,
mathematically identical):
  * scores:  S = (HQ)(HK)^T = H (QK^T) H^T  -> precompute M = QK^T, project
    once (hm = H M) and reuse H^T as the k-side operand. Kills the K proj.
  * output:  conv2 needs only W2^T O^T = ((HV) W2)^T E^T = Z^T E^T with
    Z^T = (V W2)^T H^T. O / HV are never materialized. Kills the V proj and
    the big attn*V matmul.
E^T is produced by DMA-engine XBAR transposes (16x128 tiles) instead of PE
transposes, freeing the TensorEngine. The conv2 tap scatter+sum runs as nine
shifted-window accumulates on the Pool engine, so the PE never waits on it.
"""

import numpy as np

E = 4          # batch elements per core
NCORES = 8
IMG = 32       # t = v = 32
L = IMG * IMG  # 1024 tokens
P = 512        # planes

_TAPS = [(dy, dx) for dy in range(3) for dx in range(3)]

_built = {}


def _build_nc():
    import concourse.mybir as mybir
    from concourse import bacc
    from concourse.tile import TileContext
    from concourse.masks import make_identity

    f32, f32r, bf16 = mybir.dt.float32, mybir.dt.float32r, mybir.dt.bfloat16
    AF = mybir.ActivationFunctionType
    ALU = mybir.AluOpType
    AX = mybir.AxisListType

    nc = bacc.Bacc("TRN2", target_bir_lowering=False, debug=False, num_devices=NCORES)

    i_xcol = nc.dram_tensor("xcol", [E, 9, L], f32, kind="ExternalInput")
    i_w1 = nc.dram_tensor("W1c", [9, P], f32, kind="ExternalInput")
    i_qm = nc.dram_tensor("Qm", [128, 4, P], f32, kind="ExternalInput")   # M = Q K^T
    i_vw2 = nc.dram_tensor("VW2", [128, 4, 9], f32, kind="ExternalInput")  # V @ W2
    i_w2 = nc.dram_tensor("W2m", [128, 4, 9], f32, kind="ExternalInput")
    i_b1 = nc.dram_tensor("b1v", [128, 4], f32, kind="ExternalInput")
    o_out = nc.dram_tensor("out", [E, L], f32, kind="ExternalOutput")

    with TileContext(nc) as tc:
        with (
            tc.tile_pool(name="wts", bufs=1) as wts,
            tc.tile_pool(name="stg", bufs=1) as stg,
            tc.tile_pool(name="xp", bufs=2) as xp,
            tc.tile_pool(name="hp", bufs=2) as hp,
            tc.tile_pool(name="qp", bufs=2) as qp,
            tc.tile_pool(name="ep", bufs=2) as ep,
            tc.tile_pool(name="xm", bufs=3) as xm,
            tc.tile_pool(name="msc", bufs=2) as msc,
            tc.tile_pool(name="pmm", bufs=3, space="PSUM") as pmm,
            tc.tile_pool(name="pz", bufs=2, space="PSUM") as pz,
        ):
            # ---- weights / constants (persistent, loaded once) ----
            def load_r(name, src_ap, shape):
                stage = stg.tile(shape, f32, tag="wstage")
                nc.sync.dma_start(stage[:], src_ap)
                dst = wts.tile(shape, f32r, tag=name)
                nc.vector.tensor_copy(dst[:], stage[:])
                return dst

            w1c = load_r("w1c", i_w1.ap(), [9, P])
            vw2 = load_r("vw2", i_vw2.ap(), [128, 4, 9])
            w2f = load_r("w2f", i_w2.ap(), [128, 4, 9])

            qm = wts.tile([128, 4, P], f32r, tag="qm", name="qm")
            for dk in range(4):
                stage = stg.tile([128, 1, P], f32, tag="wstage4", name=f"qms{dk}")
                nc.sync.dma_start(stage[:], i_qm.ap()[:, dk:dk + 1, :])
                nc.vector.tensor_copy(qm[:, dk:dk + 1, :], stage[:])

            b1t = wts.tile([128, 4], f32)
            nc.sync.dma_start(b1t[:], i_b1.ap())
            ident = wts.tile([128, 128], f32)
            make_identity(nc, ident[:])

            prefetch = {}
            xcf0 = xp.tile([9, L], f32, tag="xcolf", name="xcf0")
            nc.sync.dma_start(xcf0[:], i_xcol.ap()[0])
            prefetch[0] = xcf0

            def conv1_hm(e):
                """conv1 + relu -> ht [ch, tok]; hm projection -> hmT [n, tok]."""
                xcf = prefetch.pop(e, None)
                if xcf is None:
                    xcf = xp.tile([9, L], f32, tag="xcolf")
                    nc.sync.dma_start(xcf[:], i_xcol.ap()[e])
                xc = xp.tile([9, L], f32r, tag="xcol")
                nc.scalar.copy(xc[:], xcf[:])
                ht = hp.tile([128, 4, L], f32r, tag="H")
                for ck in range(4):
                    ps = pmm.tile([128, 1024], f32, tag="pmm")
                    for lg in range(2):
                        nc.tensor.matmul(
                            ps[:, lg * 512:(lg + 1) * 512],
                            w1c[:, ck * 128:(ck + 1) * 128],
                            xc[:, lg * 512:(lg + 1) * 512],
                            start=True, stop=True,
                        )
                    nc.scalar.activation(
                        ht[:, ck, :], ps[:], AF.Relu, bias=b1t[:, ck:ck + 1]
                    )
                hmT = qp.tile([128, 4, L], f32r, tag="hmT")
                for nck in range(4):
                    ps = pmm.tile([128, 1024], f32, tag="pmm")
                    for dk in range(4):
                        for lg in range(2):
                            nc.tensor.matmul(
                                ps[:, lg * 512:(lg + 1) * 512],
                                qm[:, dk, nck * 128:(nck + 1) * 128],
                                ht[:, dk, lg * 512:(lg + 1) * 512],
                                start=(dk == 0), stop=(dk == 3),
                            )
                    if nck % 2 == 0:
                        nc.scalar.copy(hmT[:, nck, :], ps[:])
                    else:
                        nc.vector.tensor_copy(hmT[:, nck, :], ps[:])
                return ht, hmT

            def attention(e, ht, hmT):
                """scores in M-layout, exact-row-max softmax numerator; E^T via
                DMA XBAR transposes."""
                nmcol = msc.tile([128, 8], f32, tag="nmcol")
                rscol = msc.tile([128, 8], f32, tag="rscol")
                et = ep.tile([128, 8, L], bf16, tag="eT")
                for lc in range(8):
                    ps = pmm.tile([128, 1024], f32, tag="pmm")
                    for ncx in range(4):
                        for mg in range(2):
                            nc.tensor.matmul(
                                ps[:, mg * 512:(mg + 1) * 512],
                                hmT[:, ncx, lc * 128:(lc + 1) * 128],
                                ht[:, ncx, mg * 512:(mg + 1) * 512],
                                start=(ncx == 0), stop=(ncx == 3),
                            )
                    nc.vector.tensor_reduce(
                        nmcol[:, lc:lc + 1], ps[:], axis=AX.X, op=ALU.max, negate=True
                    )
                    expm = xm.tile([128, 1024], bf16, tag="expM")
                    nc.scalar.activation(
                        expm[:], ps[:], AF.Exp,
                        bias=nmcol[:, lc:lc + 1],
                        accum_out=rscol[:, lc:lc + 1],
                    )
                    nc.sync.dma_start_transpose(
                        et[:, :, lc * 128:(lc + 1) * 128], expm[:]
                    )
                return et, rscol

            def tail(e, ht, hmT, et, rscol):
                # ---- Z^T = (V W2)^T H^T -> token-major z via XBAR ----
                zTs = msc.tile([16, L], bf16, tag="zTs")
                nc.gpsimd.memset(zTs[:], 0.0)
                for lg in range(2):
                    sl = slice(lg * 512, (lg + 1) * 512)
                    psz = pz.tile([9, 512], f32, tag="pz9", name="psz")
                    for dk in range(4):
                        nc.tensor.matmul(
                            psz[:], vw2[:, dk, :], ht[:, dk, sl],
                            start=(dk == 0), stop=(dk == 3),
                        )
                    nc.scalar.copy(zTs[0:9, sl], psz[:])
                z = msc.tile([128, 8, 16], bf16, tag="z")
                nc.sync.dma_start_transpose(z[:], zTs[:])

                # ---- p9h = W2^T H^T, parked in SBUF ----
                p9hs = msc.tile([9, L], f32, tag="p9hs")
                for lg in range(2):
                    sl = slice(lg * 512, (lg + 1) * 512)
                    psh = pz.tile([9, 512], f32, tag="pz9", name="psh")
                    for ck in range(4):
                        nc.tensor.matmul(
                            psh[:], w2f[:, ck, :], ht[:, ck, sl],
                            start=(ck == 0), stop=(ck == 3),
                        )
                    nc.vector.tensor_copy(p9hs[:, sl], psh[:])

                # ---- reciprocal rowsums -> per-q row [1, L] -> 9 partitions ----
                rcol = msc.tile([128, 8], f32, tag="rcol")
                nc.vector.reciprocal(rcol[:], rscol[:])
                pt = pz.tile([8, 128], f32, tag="pz9", name="pt")
                nc.tensor.transpose(pt[:], rcol[:], ident[:])
                rc8 = msc.tile([8, 128], f32, tag="rc8")
                nc.vector.tensor_copy(rc8[:], pt[:])
                rcc = msc.tile([1, L], f32, tag="rcc")
                for c in range(8):
                    nc.sync.dma_start(rcc[0:1, 128 * c:128 * (c + 1)], rc8[c:c + 1, :])
                rbc9 = msc.tile([9, L], f32, tag="rbc9")
                nc.gpsimd.partition_broadcast(rbc9[:], rcc[0:1, :])

                # next element's PE prologue goes ahead of p9o so the PE stays
                # fed while this element's E^T transposes drain on the DMA side
                nxt = conv1_hm(e + 1) if e + 1 < E else None

                # ---- p9o = Z^T E^T; normalize and add the h-side taps ----
                p9e = msc.tile([9, L], f32, tag="p9e")
                for lg in range(2):
                    sl = slice(lg * 512, (lg + 1) * 512)
                    pso = pz.tile([9, 512], f32, tag="pz9", name="pso")
                    for kc in range(8):
                        nc.tensor.matmul(
                            pso[:], z[:, kc, 0:9], et[:, kc, sl],
                            start=(kc == 0), stop=(kc == 7),
                        )
                    nc.vector.tensor_tensor(p9e[:, sl], pso[:], rbc9[:, sl], ALU.mult)
                    nc.vector.tensor_tensor(p9e[:, sl], p9e[:, sl], p9hs[:, sl], ALU.add)

                # ---- conv2 tap accumulation: center-tap copy then eight
                # shifted-window accumulating SWDGE DMAs (same queue -> FIFO;
                # byte-addressed so the odd partition bases are fine) ----
                acc = msc.tile([1, L], f32, tag="acc")
                accv = acc[:].rearrange("o (r c) -> o r c", c=IMG)
                nc.gpsimd.dma_start(acc[0:1, :], p9e[4:5, :])
                for j, (dy, dx) in enumerate(_TAPS):
                    if j == 4:
                        continue
                    r0, r1 = max(0, 1 - dy), min(IMG, IMG + 1 - dy)
                    c0, c1 = max(0, 1 - dx), min(IMG, IMG + 1 - dx)
                    srcw = p9e[j:j + 1, :].rearrange("o (r c) -> o r c", c=IMG)[
                        :, r0 + dy - 1:r1 + dy - 1, c0 + dx - 1:c1 + dx - 1
                    ]
                    nc.gpsimd.dma_start(
                        accv[:, r0:r1, c0:c1], srcw, accum_op=ALU.add
                    )
                nc.sync.dma_start(o_out.ap()[e:e + 1, :], acc[0:1, :])
                return nxt

            cur = conv1_hm(0)
            for e in range(E):
                et, rscol = attention(e, *cur)
                cur = tail(e, *cur, et, rscol)

    nc.compile()
    return nc


def _host_prep(x, W1, b1, Q, K, V, W2, b2):
    B = x.shape[0] * x.shape[1]
    xf = np.ascontiguousarray(x, np.float32).reshape(B, IMG, IMG)
    xpad = np.zeros((B, IMG + 2, IMG + 2), np.float32)
    xpad[:, 1:-1, 1:-1] = xf
    xcol = np.empty((B, 9, L), np.float32)
    for j, (dy, dx) in enumerate(_TAPS):
        xcol[:, j] = xpad[:, dy:dy + IMG, dx:dx + IMG].reshape(B, L)
    w1c = np.ascontiguousarray(np.asarray(W1, np.float32).reshape(P, 9).T)
    Qf = np.asarray(Q, np.float32)
    Kf = np.asarray(K, np.float32)
    Vf = np.asarray(V, np.float32)
    W2f = np.asarray(W2, np.float32).reshape(P, 9)
    M = Qf @ Kf.T
    VW2 = Vf @ W2f
    qm = np.ascontiguousarray(M.reshape(4, 128, P).transpose(1, 0, 2))
    vw2 = np.ascontiguousarray(VW2.reshape(4, 128, 9).transpose(1, 0, 2))
    w2m = np.ascontiguousarray(W2f.reshape(4, 128, 9).transpose(1, 0, 2))
    b1v = np.ascontiguousarray(np.asarray(b1, np.float32).reshape(4, 128).T)
    return xcol, w1c, qm, vw2, w2m, b1v


def kernel(x, W1, b1, Q, K, V, W2, b2):
    from concourse.bass_utils import run_bass_kernel_spmd

    xcol, w1c, qm, vw2, w2m, b1v = _host_prep(x, W1, b1, Q, K, V, W2, b2)
    if "nc" not in _built:
        _built["nc"] = _build_nc()
    nc = _built["nc"]
    in_maps = []
    for c in range(NCORES):
        in_maps.append({
            "xcol": np.ascontiguousarray(xcol[E * c:E * (c + 1)]),
            "W1c": w1c, "Qm": qm, "VW2": vw2,
            "W2m": w2m, "b1v": b1v,
        })
    res = run_bass_kernel_spmd(nc, in_maps, core_ids=list(range(NCORES)))
    full = np.concatenate([res.results[c]["out"] for c in range(NCORES)], axis=0)
    full = full + np.float32(np.asarray(b2, np.float32).reshape(())[()])
    return np.ascontiguousarray(
        full.reshape(x.shape[0], x.shape[1], IMG, IMG).astype(np.float32)
    )
